# revision 31
# baseline (speedup 1.0000x reference)
"""Multi-headed self-attention on 8 Trainium2 NeuronCores (Bass/Tile).

Problem: B=4, S=2048, HID=1024, H=16 heads (D=64).
Returns (h, scores) like the reference:
    qp = q @ Wq.T + bq ; kp, vp likewise
    scores = softmax(Qh @ Kh.T / sqrt(D) - 10000*(1-mask))   [B,H,S,S]
    h      = scores @ Vh  (heads merged)                      [B,S,HID]

Sharding: core c handles batch b=c//2 and head-group g=c%2 (8 heads,
512 channels).  Data-parallel over batch x tensor-parallel over heads.

Device kernel (per core), computed entirely in transposed layouts so no
on-chip transposes are needed:
  - inputs  qT/kT/vT [1024, 2048] (hidden-on-partitions) and weight slices
    wqT/wkT/wvT [1024, 512]; 1/sqrt(D) folded into wqT on the host.  (If
    biases are nonzero, a 9th contraction tile with a ones-row/bias-row is
    added -- the homogeneous-coordinate fold.)
  - projections: QpT,KpT -> [512c, 2048s] channels-on-partitions;
    Vp -> [2048s, 512c] seq-on-partitions, stored per-head with an appended
    ones column (for softmax row sums).
  - attention runs in steps over (head-pair, q-window of 512, k-tile):
    scoresT[k,q] = Kh @ Qh^T for both heads of the pair lands in ONE
    [128,1024] PSUM tile (even head cols 0:512, odd head cols 512:1024;
    d=64 contraction at partition offsets 0/64 -> the two matmuls run
    concurrently on disjoint PE row groups).  One wide ScalarE ACTIVATE
    evacuates it with exp(); the exp tile feeds both AV matmuls
    (lhsT = [Vh | 1], accumulating [h^T_unnorm ; Z] in [65,512] PSUM) one
    step later (software pipeline) and is DMAed to HBM unnormalized.
    The phase is exp()-paced on ScalarE; projection matmul groups for the
    next head-pair (and all of Vp during the first pair) are interleaved
    into the stream, borrowing the qk PSUM slots.
  - host normalizes by Z and transposes back to the reference layout.
"""

import os
import numpy as np

B, S, HID, H = 4, 2048, 1024, 16
D = HID // H            # 64
N_CORES = 8
HPC = 8                 # heads per core
CPC = 512               # channels per core
QW = 512                # q-window per attention step
NQH = S // QW           # 4
NCHUNK = 512            # matmul moving-operand free dim

_DT_NAME = os.environ.get("ATTN_DT", "bf16")  # fp16 | bf16 | fp32

_CACHE = {}


def _np_dt():
    if _DT_NAME == "fp32":
        return np.float32
    if _DT_NAME == "fp16":
        return np.float16
    import ml_dtypes

    return ml_dtypes.bfloat16


def _build_nc(n_it):
    import concourse.mybir as mybir
    import concourse.tile as tile
    from concourse import bacc

    dt = {
        "fp32": mybir.dt.float32,
        "fp16": mybir.dt.float16,
        "bf16": mybir.dt.bfloat16,
    }[_DT_NAME]
    f32 = mybir.dt.float32
    Exp = mybir.ActivationFunctionType.Exp

    nc = bacc.Bacc(
        "TRN2", target_bir_lowering=False, debug=False, num_devices=N_CORES
    )
    rows = n_it * 128

    qT = nc.dram_tensor("qT", [rows, S], dt, kind="ExternalInput").ap()
    kT = nc.dram_tensor("kT", [rows, S], dt, kind="ExternalInput").ap()
    vT = nc.dram_tensor("vT", [rows, S], dt, kind="ExternalInput").ap()
    wqT = nc.dram_tensor("wqT", [rows, CPC], dt, kind="ExternalInput").ap()
    wkT = nc.dram_tensor("wkT", [rows, CPC], dt, kind="ExternalInput").ap()
    wvT = nc.dram_tensor("wvT", [rows, CPC], dt, kind="ExternalInput").ap()
    # tile-major layouts so every DMA-out is one contiguous-ish block
    expT = nc.dram_tensor(
        "expT", [HPC // 2, NQH, 16, 128, 2, QW], dt, kind="ExternalOutput"
    ).ap()
    hzT = nc.dram_tensor(
        "hzT", [HPC // 2, NQH, 128, QW], f32, kind="ExternalOutput"
    ).ap()

    with (
        tile.TileContext(nc) as tc,
        tc.tile_pool(name="pout", bufs=1) as pout,
        tc.tile_pool(name="xqk", bufs=1) as xqk,
        tc.tile_pool(name="wqk", bufs=1) as wqk,
        tc.tile_pool(name="mmps", bufs=1, space="PSUM") as mmps,
    ):
        QpT = pout.tile([128, 4, S], dt)          # [c%128, c//128, s]
        KpT = pout.tile([128, 4, S], dt)
        Vp = pout.tile([128, 16, HPC, D], dt)     # [s%128, s//128, head, d]

        # input loads on the ScalarE HWDGE ring: separate from the output
        # (SP) ring, fast first-byte, and ScalarE is idle at kernel start.
        # One batched DMA per tensor keeps the issue cost off the ramp.
        def load_all(pool, ap, n_free, tag):
            t = pool.tile([128, n_it, n_free], dt, tag=tag, name=tag)
            for it in range(n_it):
                nc.sync.dma_start(
                    out=t[:, it, :], in_=ap[it * 128:(it + 1) * 128, :]
                )
            return [t[:, it, :] for it in range(n_it)]

        wv = load_all(wqk, wvT, CPC, "wv")
        xvt = load_all(xqk, vT, S, "xv")
        wq = load_all(wqk, wqT, CPC, "wq")
        xq = load_all(xqk, qT, S, "xq")
        wk = load_all(wqk, wkT, CPC, "wk")
        xk = load_all(xqk, kT, S, "xk")

        # projection matmul groups (borrow the "qk" psum slots)
        def qk_proj_chunks(dst, wts, xts, ct, sc):
            # emission-units of ~3 matmuls so a group never stalls the
            # QK/exp pipeline for a whole accumulation group
            ps = mmps.tile([128, NCHUNK], f32, tag="qk", bufs=3, name="pp")

            def unit(i0, i1):
                def emit():
                    for it in range(i0, i1):
                        nc.tensor.matmul(
                            ps[:],
                            lhsT=wts[it][:, ct * 128:(ct + 1) * 128],
                            rhs=xts[it][:, sc * NCHUNK:(sc + 1) * NCHUNK],
                            start=(it == 0),
                            stop=(it == n_it - 1),
                        )
                    if i1 == n_it:
                        nc.vector.tensor_copy(
                            dst[:, ct, sc * NCHUNK:(sc + 1) * NCHUNK], ps[:]
                        )
                return emit

            bounds = [0, 3, 6, n_it]
            return [unit(bounds[i], bounds[i + 1]) for i in range(3)]

        def qk_proj_group(dst, wts, xts, ct, sc):
            for u in qk_proj_chunks(dst, wts, xts, ct, sc):
                u()

        def v_proj_group(st):
            def emit():
                tag = "av" if st % 2 else "qk"
                ps = mmps.tile(
                    [128, NCHUNK], f32, tag=tag, bufs=2 if st % 2 else 3, name="pp"
                )
                for it in range(n_it):
                    nc.tensor.matmul(
                        ps[:],
                        lhsT=xvt[it][:, st * 128:(st + 1) * 128],
                        rhs=wv[it][:],
                        start=(it == 0),
                        stop=(it == n_it - 1),
                    )
                nc.vector.tensor_copy(
                    Vp[:, st, :, :], ps.rearrange("p (h d) -> p h d", h=HPC)
                )
            return emit

        # prologue: all Vp groups (v arrives first on the wire), then the
        # first q/k projection slices; other slices are urgent sprinkles
        for st in range(8):
            v_proj_group(st)()
        qk_proj_group(QpT, wq, xq, 0, 0)
        qk_proj_group(KpT, wk, xk, 0, 0)

        _exp_ctx = tc.tile_pool(name="expool", bufs=16)
        expool = _exp_ctx.__enter__()
        _hz_ctx = tc.tile_pool(name="hzpool", bufs=3)
        hzpool = _hz_ctx.__enter__()

        # Deep software pipeline: exp tiles wait in `pending` until their AV
        # matmuls are flushed.  During hp=0 the pipeline runs ~14 k-tiles
        # deep so attention can start long before Vp exists (v is last on
        # the wire); later head-pairs run 4 deep.
        pending = []  # entries: (et, kt, av_tile, hp, qh)

        def flush_one():
            et, kt, av, fhp, fqh = pending.pop(0)
            for j in range(2):
                nc.tensor.matmul(
                    av[j * D:(j + 1) * D, :],
                    lhsT=Vp[:, kt, 2 * fhp + j, :],
                    rhs=et[:, j * QW:(j + 1) * QW],
                    start=(kt == 0),
                    stop=(kt == 15),
                )
            if kt == 15:
                hz = hzpool.tile([128, QW], f32, tag="hz")
                nc.vector.tensor_copy(hz[:], av[:])
                nc.sync.dma_start(out=hzT[fhp, fqh], in_=hz[:])

        for hp in range(HPC // 2):
            sched = {}

            def put(step, unit):
                sched.setdefault(step, []).append(unit)

            if hp == 0:
                for st in range(8, 16):  # Vp tail: first needed at kt-step 8
                    put(st - 8, v_proj_group(st))
                u = 0
                for sc in range(1, 4):  # KpT slices: needed from kt=4 on
                    for c in qk_proj_chunks(KpT, wk, xk, 0, sc):
                        put(u, c)
                        u += 1
                for sc in range(1, 4):  # QpT slices: needed from qh=sc on
                    for c in qk_proj_chunks(QpT, wq, xq, 0, sc):
                        put(u, c)
                        u += 1
                nxt = u
            else:
                nxt = 2
            if hp + 1 < HPC // 2:
                units = []
                for sc in range(4):
                    units += qk_proj_chunks(QpT, wq, xq, hp + 1, sc)
                    units += qk_proj_chunks(KpT, wk, xk, hp + 1, sc)
                span = NQH * 16 - nxt
                for i, c in enumerate(units):
                    put(nxt + (i * span) // len(units), c)
            depth = 4
            step = 0
            for qh in range(NQH):
                av = mmps.tile([128, QW], f32, tag="av", bufs=2, name="av")
                for kt2 in range(0, 16, 2):
                    qks = []
                    for kt in (kt2, kt2 + 1):
                        qk = mmps.tile(
                            [128, 2 * QW], f32, tag="qk", bufs=3, name="qk"
                        )
                        for j in range(2):
                            po = j * 64
                            nc.tensor.matmul(
                                qk[:, j * QW:(j + 1) * QW],
                                lhsT=KpT[po:po + 64, hp, kt * 128:(kt + 1) * 128],
                                rhs=QpT[po:po + 64, hp, qh * QW:(qh + 1) * QW],
                                start=True,
                                stop=True,
                            )
                        qks.append((qk, kt))
                    while len(pending) > depth:
                        flush_one()
                    for qk, kt in qks:
                        et = expool.tile([128, 2 * QW], dt, tag="exp")
                        nc.scalar.activation(et[:], qk[:], Exp)
                        nc.sync.dma_start(
                            out=expT[hp, qh, kt],
                            in_=et.rearrange("p (j w) -> p j w", j=2),
                        )
                        pending.append((et, kt, av, hp, qh))
                    for kt in (kt2, kt2 + 1):
                        for unit in sched.pop(step, []):
                            unit()
                        step += 1
            for s in sorted(sched):
                for unit in sched[s]:
                    unit()
            sched.clear()
        while pending:
            flush_one()

        _hz_ctx.__exit__(None, None, None)
        _exp_ctx.__exit__(None, None, None)

    nc.compile()
    return nc


def get_nc(n_it=8):
    key = ("nc", n_it)
    if key not in _CACHE:
        _CACHE[key] = _build_nc(n_it)
    return _CACHE[key]


def prepare_in_maps(q, k, v, Wq, bq, Wk, bk, Wv, bv, **_unused):
    dt = _np_dt()
    q, k, v = np.asarray(q), np.asarray(k), np.asarray(v)
    Wq, Wk, Wv = np.asarray(Wq), np.asarray(Wk), np.asarray(Wv)
    bq, bk, bv = np.asarray(bq), np.asarray(bk), np.asarray(bv)
    scale = 1.0 / np.sqrt(np.float32(D))
    zero_bias = not (np.any(bq) or np.any(bk) or np.any(bv))
    n_it = 8 if zero_bias else 9
    rows = n_it * 128

    def aug_x(xb):  # [S, HID] -> [rows, S]
        out = np.zeros((rows, S), dtype=dt)
        out[:HID] = xb.T.astype(dt)
        if n_it == 9:
            out[HID] = 1.0
        return out

    def aug_w(W, b, g, sc):  # -> [rows, CPC]
        out = np.zeros((rows, CPC), dtype=dt)
        sl = slice(g * CPC, (g + 1) * CPC)
        out[:HID] = (W[sl, :].T * sc).astype(dt)
        if n_it == 9:
            out[HID] = (b[sl] * sc).astype(dt)
        return out

    xcache = {b: (aug_x(q[b]), aug_x(k[b]), aug_x(v[b])) for b in range(B)}
    wcache = {
        g: (
            aug_w(Wq, bq, g, scale),
            aug_w(Wk, bk, g, 1.0),
            aug_w(Wv, bv, g, 1.0),
        )
        for g in range(2)
    }
    in_maps = []
    for c in range(N_CORES):
        b, g = c // 2, c % 2
        qTa, kTa, vTa = xcache[b]
        wqTa, wkTa, wvTa = wcache[g]
        in_maps.append(
            {"qT": qTa, "kT": kTa, "vT": vTa, "wqT": wqTa, "wkT": wkTa, "wvT": wvTa}
        )
    return in_maps, n_it


def assemble(results):
    h = np.empty((B, S, HID), np.float32)
    scores = np.empty((B, H, S, S), np.float32)
    for c in range(N_CORES):
        b, g = c // 2, c % 2
        hzt = np.asarray(results[c]["hzT"], np.float32)  # [HPC//2, NQH, 128, QW]
        hz = hzt.transpose(0, 2, 1, 3).reshape(HPC // 2, 128, S)
        expTc = results[c]["expT"]          # [HPC//2, NQH, 16, 128, 2, QW]
        for hl in range(HPC):
            hp, j = hl // 2, hl % 2
            e = np.asarray(expTc[hp, :, :, :, j], np.float32)  # [NQH,16,128,QW]
            e_kq = e.transpose(1, 2, 0, 3).reshape(S, S)    # [k, q]
            Z = e_kq.sum(axis=0)                            # [q]
            scores[b, g * HPC + hl] = (e_kq / Z[None, :]).T
            h[b, :, g * CPC + hl * D:g * CPC + (hl + 1) * D] = (
                hz[hp, j * D:(j + 1) * D, :] / Z[None, :]
            ).T
    return h, scores


def run_spmd(in_maps, n_it=8, trace=False, tmpdir=None):
    from concourse.bass_utils import run_bass_kernel_spmd

    return run_bass_kernel_spmd(
        get_nc(n_it), in_maps, list(range(N_CORES)), trace=trace, tmpdir=tmpdir
    )


def kernel(**inputs):
    in_maps, n_it = prepare_in_maps(**inputs)
    res = run_spmd(in_maps, n_it)
    return assemble(res.results)


# revision 32
# speedup vs baseline: 1.0254x; 1.0254x over previous
"""Multi-headed self-attention on 8 Trainium2 NeuronCores (Bass/Tile).

Problem: B=4, S=2048, HID=1024, H=16 heads (D=64).
Returns (h, scores) like the reference:
    qp = q @ Wq.T + bq ; kp, vp likewise
    scores = softmax(Qh @ Kh.T / sqrt(D) - 10000*(1-mask))   [B,H,S,S]
    h      = scores @ Vh  (heads merged)                      [B,S,HID]

Sharding: core c handles batch b=c//2 and head-group g=c%2 (8 heads,
512 channels).  Data-parallel over batch x tensor-parallel over heads.

Device kernel (per core), computed entirely in transposed layouts so no
on-chip transposes are needed:
  - inputs  qT/kT/vT [1024, 2048] (hidden-on-partitions) and weight slices
    wqT/wkT/wvT [1024, 512]; 1/sqrt(D) folded into wqT on the host.  (If
    biases are nonzero, a 9th contraction tile with a ones-row/bias-row is
    added -- the homogeneous-coordinate fold.)
  - projections: QpT,KpT -> [512c, 2048s] channels-on-partitions;
    Vp -> [2048s, 512c] seq-on-partitions, stored per-head with an appended
    ones column (for softmax row sums).
  - attention runs in steps over (head-pair, q-window of 512, k-tile):
    scoresT[k,q] = Kh @ Qh^T for both heads of the pair lands in ONE
    [128,1024] PSUM tile (even head cols 0:512, odd head cols 512:1024;
    d=64 contraction at partition offsets 0/64 -> the two matmuls run
    concurrently on disjoint PE row groups).  One wide ScalarE ACTIVATE
    evacuates it with exp(); the exp tile feeds both AV matmuls
    (lhsT = [Vh | 1], accumulating [h^T_unnorm ; Z] in [65,512] PSUM) one
    step later (software pipeline) and is DMAed to HBM unnormalized.
    The phase is exp()-paced on ScalarE; projection matmul groups for the
    next head-pair (and all of Vp during the first pair) are interleaved
    into the stream, borrowing the qk PSUM slots.
  - host normalizes by Z and transposes back to the reference layout.
"""

import os
import numpy as np

B, S, HID, H = 4, 2048, 1024, 16
D = HID // H            # 64
N_CORES = 8
HPC = 8                 # heads per core
CPC = 512               # channels per core
QW = 512                # q-window per attention step
NQH = S // QW           # 4
NCHUNK = 512            # matmul moving-operand free dim

_DT_NAME = os.environ.get("ATTN_DT", "bf16")  # fp16 | bf16 | fp32

_CACHE = {}


def _np_dt():
    if _DT_NAME == "fp32":
        return np.float32
    if _DT_NAME == "fp16":
        return np.float16
    import ml_dtypes

    return ml_dtypes.bfloat16


def _build_nc(n_it):
    import concourse.mybir as mybir
    import concourse.tile as tile
    from concourse import bacc

    dt = {
        "fp32": mybir.dt.float32,
        "fp16": mybir.dt.float16,
        "bf16": mybir.dt.bfloat16,
    }[_DT_NAME]
    f32 = mybir.dt.float32
    Exp = mybir.ActivationFunctionType.Exp

    nc = bacc.Bacc(
        "TRN2", target_bir_lowering=False, debug=False, num_devices=N_CORES
    )
    rows = n_it * 128

    qT = nc.dram_tensor("qT", [rows, S], dt, kind="ExternalInput").ap()
    kT = nc.dram_tensor("kT", [rows, S], dt, kind="ExternalInput").ap()
    vT = nc.dram_tensor("vT", [rows, S], dt, kind="ExternalInput").ap()
    wqT = nc.dram_tensor("wqT", [rows, CPC], dt, kind="ExternalInput").ap()
    wkT = nc.dram_tensor("wkT", [rows, CPC], dt, kind="ExternalInput").ap()
    wvT = nc.dram_tensor("wvT", [rows, CPC], dt, kind="ExternalInput").ap()
    # tile-major layouts so every DMA-out is one contiguous-ish block
    expT = nc.dram_tensor(
        "expT", [HPC // 2, NQH, 16, 128, 2, QW], dt, kind="ExternalOutput"
    ).ap()
    hzT = nc.dram_tensor(
        "hzT", [HPC // 2, NQH, 128, QW], f32, kind="ExternalOutput"
    ).ap()

    with (
        tile.TileContext(nc) as tc,
        tc.tile_pool(name="pout", bufs=1) as pout,
        tc.tile_pool(name="xqk", bufs=1) as xqk,
        tc.tile_pool(name="wqk", bufs=1) as wqk,
        tc.tile_pool(name="mmps", bufs=1, space="PSUM") as mmps,
    ):
        QpT = pout.tile([128, 4, S], dt)          # [c%128, c//128, s]
        KpT = pout.tile([128, 4, S], dt)
        Vp = pout.tile([128, 16, HPC, D], dt)     # [s%128, s//128, head, d]

        # input loads on the ScalarE HWDGE ring: separate from the output
        # (SP) ring, fast first-byte, and ScalarE is idle at kernel start.
        # One batched DMA per tensor keeps the issue cost off the ramp.
        def load_all(pool, ap, n_free, tag):
            t = pool.tile([128, n_it, n_free], dt, tag=tag, name=tag)
            for it in range(n_it):
                nc.sync.dma_start(
                    out=t[:, it, :], in_=ap[it * 128:(it + 1) * 128, :]
                )
            return [t[:, it, :] for it in range(n_it)]

        wv = load_all(wqk, wvT, CPC, "wv")
        xvt = load_all(xqk, vT, S, "xv")
        wq = load_all(wqk, wqT, CPC, "wq")
        xq = load_all(xqk, qT, S, "xq")
        wk = load_all(wqk, wkT, CPC, "wk")
        xk = load_all(xqk, kT, S, "xk")

        # projection matmul groups (borrow the "qk" psum slots)
        def qk_proj_chunks(dst, wts, xts, ct, sc):
            # emission-units of ~3 matmuls so a group never stalls the
            # QK/exp pipeline for a whole accumulation group
            ps = mmps.tile([128, NCHUNK], f32, tag="qk", bufs=3, name="pp")

            def unit(i0, i1):
                def emit():
                    for it in range(i0, i1):
                        nc.tensor.matmul(
                            ps[:],
                            lhsT=wts[it][:, ct * 128:(ct + 1) * 128],
                            rhs=xts[it][:, sc * NCHUNK:(sc + 1) * NCHUNK],
                            start=(it == 0),
                            stop=(it == n_it - 1),
                        )
                    if i1 == n_it:
                        nc.vector.tensor_copy(
                            dst[:, ct, sc * NCHUNK:(sc + 1) * NCHUNK], ps[:]
                        )
                return emit

            bounds = [0, 3, 6, n_it]
            return [unit(bounds[i], bounds[i + 1]) for i in range(3)]

        def qk_proj_group(dst, wts, xts, ct, sc):
            for u in qk_proj_chunks(dst, wts, xts, ct, sc):
                u()

        def v_proj_group(st):
            def emit():
                tag = "av" if st % 2 else "qk"
                ps = mmps.tile(
                    [128, NCHUNK], f32, tag=tag, bufs=2 if st % 2 else 3, name="pp"
                )
                for it in range(n_it):
                    nc.tensor.matmul(
                        ps[:],
                        lhsT=xvt[it][:, st * 128:(st + 1) * 128],
                        rhs=wv[it][:],
                        start=(it == 0),
                        stop=(it == n_it - 1),
                    )
                nc.vector.tensor_copy(
                    Vp[:, st, :, :], ps.rearrange("p (h d) -> p h d", h=HPC)
                )
            return emit

        # prologue: all Vp groups (v arrives first on the wire), then the
        # first q/k projection slices; other slices are urgent sprinkles
        for st in range(16):
            v_proj_group(st)()
        qk_proj_group(QpT, wq, xq, 0, 0)
        qk_proj_group(KpT, wk, xk, 0, 0)

        _exp_ctx = tc.tile_pool(name="expool", bufs=16)
        expool = _exp_ctx.__enter__()
        _hz_ctx = tc.tile_pool(name="hzpool", bufs=3)
        hzpool = _hz_ctx.__enter__()

        # Deep software pipeline: exp tiles wait in `pending` until their AV
        # matmuls are flushed.  During hp=0 the pipeline runs ~14 k-tiles
        # deep so attention can start long before Vp exists (v is last on
        # the wire); later head-pairs run 4 deep.
        pending = []  # entries: (et, kt, av_tile, hp, qh)

        def flush_one():
            et, kt, av, fhp, fqh = pending.pop(0)
            for j in range(2):
                nc.tensor.matmul(
                    av[j * D:(j + 1) * D, :],
                    lhsT=Vp[:, kt, 2 * fhp + j, :],
                    rhs=et[:, j * QW:(j + 1) * QW],
                    start=(kt == 0),
                    stop=(kt == 15),
                )
            if kt == 15:
                hz = hzpool.tile([128, QW], f32, tag="hz")
                nc.vector.tensor_copy(hz[:], av[:])
                nc.sync.dma_start(out=hzT[fhp, fqh], in_=hz[:])

        for hp in range(HPC // 2):
            sched = {}

            def put(step, unit):
                sched.setdefault(step, []).append(unit)

            if hp == 0:
                u = 0
                for sc in range(1, 4):  # KpT slices: needed from kt=4 on
                    for c in qk_proj_chunks(KpT, wk, xk, 0, sc):
                        put(u, c)
                        u += 1
                for sc in range(1, 4):  # QpT slices: needed from qh=sc on
                    for c in qk_proj_chunks(QpT, wq, xq, 0, sc):
                        put(u, c)
                        u += 1
                nxt = u
            else:
                nxt = 2
            if hp + 1 < HPC // 2:
                units = []
                for sc in range(4):
                    units += qk_proj_chunks(QpT, wq, xq, hp + 1, sc)
                    units += qk_proj_chunks(KpT, wk, xk, hp + 1, sc)
                span = NQH * 16 - nxt
                for i, c in enumerate(units):
                    put(nxt + (i * span) // len(units), c)
            depth = 4
            step = 0
            for qh in range(NQH):
                av = mmps.tile([128, QW], f32, tag="av", bufs=2, name="av")
                for kt2 in range(0, 16, 2):
                    qks = []
                    for kt in (kt2, kt2 + 1):
                        qk = mmps.tile(
                            [128, 2 * QW], f32, tag="qk", bufs=3, name="qk"
                        )
                        for j in range(2):
                            po = j * 64
                            nc.tensor.matmul(
                                qk[:, j * QW:(j + 1) * QW],
                                lhsT=KpT[po:po + 64, hp, kt * 128:(kt + 1) * 128],
                                rhs=QpT[po:po + 64, hp, qh * QW:(qh + 1) * QW],
                                start=True,
                                stop=True,
                            )
                        qks.append((qk, kt))
                    while len(pending) > depth:
                        flush_one()
                    for qk, kt in qks:
                        et = expool.tile([128, 2 * QW], dt, tag="exp")
                        nc.scalar.activation(et[:], qk[:], Exp)
                        nc.sync.dma_start(
                            out=expT[hp, qh, kt],
                            in_=et.rearrange("p (j w) -> p j w", j=2),
                        )
                        pending.append((et, kt, av, hp, qh))
                    for kt in (kt2, kt2 + 1):
                        for unit in sched.pop(step, []):
                            unit()
                        step += 1
            for s in sorted(sched):
                for unit in sched[s]:
                    unit()
            sched.clear()
        while pending:
            flush_one()

        _hz_ctx.__exit__(None, None, None)
        _exp_ctx.__exit__(None, None, None)

    nc.compile()
    return nc


def get_nc(n_it=8):
    key = ("nc", n_it)
    if key not in _CACHE:
        _CACHE[key] = _build_nc(n_it)
    return _CACHE[key]


def prepare_in_maps(q, k, v, Wq, bq, Wk, bk, Wv, bv, **_unused):
    dt = _np_dt()
    q, k, v = np.asarray(q), np.asarray(k), np.asarray(v)
    Wq, Wk, Wv = np.asarray(Wq), np.asarray(Wk), np.asarray(Wv)
    bq, bk, bv = np.asarray(bq), np.asarray(bk), np.asarray(bv)
    scale = 1.0 / np.sqrt(np.float32(D))
    zero_bias = not (np.any(bq) or np.any(bk) or np.any(bv))
    n_it = 8 if zero_bias else 9
    rows = n_it * 128

    def aug_x(xb):  # [S, HID] -> [rows, S]
        out = np.zeros((rows, S), dtype=dt)
        out[:HID] = xb.T.astype(dt)
        if n_it == 9:
            out[HID] = 1.0
        return out

    def aug_w(W, b, g, sc):  # -> [rows, CPC]
        out = np.zeros((rows, CPC), dtype=dt)
        sl = slice(g * CPC, (g + 1) * CPC)
        out[:HID] = (W[sl, :].T * sc).astype(dt)
        if n_it == 9:
            out[HID] = (b[sl] * sc).astype(dt)
        return out

    xcache = {b: (aug_x(q[b]), aug_x(k[b]), aug_x(v[b])) for b in range(B)}
    wcache = {
        g: (
            aug_w(Wq, bq, g, scale),
            aug_w(Wk, bk, g, 1.0),
            aug_w(Wv, bv, g, 1.0),
        )
        for g in range(2)
    }
    in_maps = []
    for c in range(N_CORES):
        b, g = c // 2, c % 2
        qTa, kTa, vTa = xcache[b]
        wqTa, wkTa, wvTa = wcache[g]
        in_maps.append(
            {"qT": qTa, "kT": kTa, "vT": vTa, "wqT": wqTa, "wkT": wkTa, "wvT": wvTa}
        )
    return in_maps, n_it


def assemble(results):
    h = np.empty((B, S, HID), np.float32)
    scores = np.empty((B, H, S, S), np.float32)
    for c in range(N_CORES):
        b, g = c // 2, c % 2
        hzt = np.asarray(results[c]["hzT"], np.float32)  # [HPC//2, NQH, 128, QW]
        hz = hzt.transpose(0, 2, 1, 3).reshape(HPC // 2, 128, S)
        expTc = results[c]["expT"]          # [HPC//2, NQH, 16, 128, 2, QW]
        for hl in range(HPC):
            hp, j = hl // 2, hl % 2
            e = np.asarray(expTc[hp, :, :, :, j], np.float32)  # [NQH,16,128,QW]
            e_kq = e.transpose(1, 2, 0, 3).reshape(S, S)    # [k, q]
            Z = e_kq.sum(axis=0)                            # [q]
            scores[b, g * HPC + hl] = (e_kq / Z[None, :]).T
            h[b, :, g * CPC + hl * D:g * CPC + (hl + 1) * D] = (
                hz[hp, j * D:(j + 1) * D, :] / Z[None, :]
            ).T
    return h, scores


def run_spmd(in_maps, n_it=8, trace=False, tmpdir=None):
    from concourse.bass_utils import run_bass_kernel_spmd

    return run_bass_kernel_spmd(
        get_nc(n_it), in_maps, list(range(N_CORES)), trace=trace, tmpdir=tmpdir
    )


def kernel(**inputs):
    in_maps, n_it = prepare_in_maps(**inputs)
    res = run_spmd(in_maps, n_it)
    return assemble(res.results)


# revision 34
# speedup vs baseline: 1.0383x; 1.0125x over previous
"""Multi-headed self-attention on 8 Trainium2 NeuronCores (Bass/Tile).

Problem: B=4, S=2048, HID=1024, H=16 heads (D=64).
Returns (h, scores) like the reference:
    qp = q @ Wq.T + bq ; kp, vp likewise
    scores = softmax(Qh @ Kh.T / sqrt(D) - 10000*(1-mask))   [B,H,S,S]
    h      = scores @ Vh  (heads merged)                      [B,S,HID]

Sharding: core c handles batch b=c//2 and head-group g=c%2 (8 heads,
512 channels).  Data-parallel over batch x tensor-parallel over heads.

Device kernel (per core), computed entirely in transposed layouts so no
on-chip transposes are needed:
  - inputs  qT/kT/vT [1024, 2048] (hidden-on-partitions) and weight slices
    wqT/wkT/wvT [1024, 512]; 1/sqrt(D) folded into wqT on the host.  (If
    biases are nonzero, a 9th contraction tile with a ones-row/bias-row is
    added -- the homogeneous-coordinate fold.)
  - projections: QpT,KpT -> [512c, 2048s] channels-on-partitions;
    Vp -> [2048s, 512c] seq-on-partitions, stored per-head.
  - attention runs in steps over (head-pair, q-window of 512, k-tile):
    scoresT[k,q] = Kh @ Qh^T for both heads of the pair lands in ONE
    [128,1024] PSUM tile (even head cols 0:512, odd head cols 512:1024;
    d=64 contraction at partition offsets 0/64 -> the two matmuls run
    concurrently on disjoint PE row groups).  One wide ScalarE ACTIVATE
    evacuates it with exp(); the bf16 exp tile is DMAed to HBM unnormalized
    and, a few steps later (software pipeline), feeds the two AV matmuls --
    col-tiled m=64 pairs accumulating both heads' h^T in one [128,512]
    PSUM bank, concurrent on disjoint PE column groups.
    The attention phase is exp()-paced on ScalarE (~99% busy); projection
    matmul groups for the next head-pair are chopped into 3-matmul units
    and interleaved into the stream, borrowing the qk PSUM slots.
  - host computes row sums Z from the returned exp matrix, normalizes,
    applies the (all-ones, per spec) mask if it ever is not, and transposes
    back to the reference layout.
"""

import os
import numpy as np

B, S, HID, H = 4, 2048, 1024, 16
D = HID // H            # 64
N_CORES = 8
HPC = 8                 # heads per core
CPC = 512               # channels per core
QW = 512                # q-window per attention step
NQH = S // QW           # 4
NCHUNK = 512            # matmul moving-operand free dim

_DT_NAME = os.environ.get("ATTN_DT", "bf16")  # fp16 | bf16 | fp32

_CACHE = {}


def _np_dt():
    if _DT_NAME == "fp32":
        return np.float32
    if _DT_NAME == "fp16":
        return np.float16
    import ml_dtypes

    return ml_dtypes.bfloat16


def _build_nc(n_it):
    import concourse.mybir as mybir
    import concourse.tile as tile
    from concourse import bacc

    dt = {
        "fp32": mybir.dt.float32,
        "fp16": mybir.dt.float16,
        "bf16": mybir.dt.bfloat16,
    }[_DT_NAME]
    f32 = mybir.dt.float32
    Exp = mybir.ActivationFunctionType.Exp

    nc = bacc.Bacc(
        "TRN2", target_bir_lowering=False, debug=False, num_devices=N_CORES
    )
    rows = n_it * 128

    qT = nc.dram_tensor("qT", [rows, S], dt, kind="ExternalInput").ap()
    kT = nc.dram_tensor("kT", [rows, S], dt, kind="ExternalInput").ap()
    vT = nc.dram_tensor("vT", [rows, S], dt, kind="ExternalInput").ap()
    wqT = nc.dram_tensor("wqT", [rows, CPC], dt, kind="ExternalInput").ap()
    wkT = nc.dram_tensor("wkT", [rows, CPC], dt, kind="ExternalInput").ap()
    wvT = nc.dram_tensor("wvT", [rows, CPC], dt, kind="ExternalInput").ap()
    # tile-major layouts so every DMA-out is one contiguous-ish block
    expT = nc.dram_tensor(
        "expT", [HPC // 2, NQH, 16, 128, 2, QW], dt, kind="ExternalOutput"
    ).ap()
    hzT = nc.dram_tensor(
        "hzT", [HPC // 2, NQH, 128, QW], f32, kind="ExternalOutput"
    ).ap()

    with (
        tile.TileContext(nc) as tc,
        tc.tile_pool(name="pout", bufs=1) as pout,
        tc.tile_pool(name="xqk", bufs=1) as xqk,
        tc.tile_pool(name="wqk", bufs=1) as wqk,
        tc.tile_pool(name="mmps", bufs=1, space="PSUM") as mmps,
    ):
        QpT = pout.tile([128, 4, S], dt)          # [c%128, c//128, s]
        KpT = pout.tile([128, 4, S], dt)
        Vp = pout.tile([128, 16, HPC, D], dt)     # [s%128, s//128, head, d]

        # per-tile input loads (HWDGE/SP ring) so projection matmuls can
        # start as their contraction tiles land; v goes first on the wire
        # because the Vp projection groups are the bulk of the ramp's PE
        # work and overlap the q/k transfers.
        def load_all(pool, ap, n_free, tag):
            t = pool.tile([128, n_it, n_free], dt, tag=tag, name=tag)
            for it in range(n_it):
                nc.sync.dma_start(
                    out=t[:, it, :], in_=ap[it * 128:(it + 1) * 128, :]
                )
            return [t[:, it, :] for it in range(n_it)]

        wv = load_all(wqk, wvT, CPC, "wv")
        xvt = load_all(xqk, vT, S, "xv")
        wq = load_all(wqk, wqT, CPC, "wq")
        xq = load_all(xqk, qT, S, "xq")
        wk = load_all(wqk, wkT, CPC, "wk")
        xk = load_all(xqk, kT, S, "xk")

        # projection matmul groups (borrow the "qk" psum slots)
        def qk_proj_chunks(dst, wts, xts, ct, sc):
            # emission-units of ~3 matmuls so a group never stalls the
            # QK/exp pipeline for a whole accumulation group
            ps = mmps.tile([128, NCHUNK], f32, tag="qk", bufs=3, name="pp")

            def unit(i0, i1):
                def emit():
                    for it in range(i0, i1):
                        nc.tensor.matmul(
                            ps[:],
                            lhsT=wts[it][:, ct * 128:(ct + 1) * 128],
                            rhs=xts[it][:, sc * NCHUNK:(sc + 1) * NCHUNK],
                            start=(it == 0),
                            stop=(it == n_it - 1),
                        )
                    if i1 == n_it:
                        nc.vector.tensor_copy(
                            dst[:, ct, sc * NCHUNK:(sc + 1) * NCHUNK], ps[:]
                        )
                return emit

            bounds = [0, 3, 6, n_it]
            return [unit(bounds[i], bounds[i + 1]) for i in range(3)]

        def qk_proj_group(dst, wts, xts, ct, sc):
            for u in qk_proj_chunks(dst, wts, xts, ct, sc):
                u()

        def v_proj_group(st):
            def emit():
                tag = "av" if st % 2 else "qk"
                ps = mmps.tile(
                    [128, NCHUNK], f32, tag=tag, bufs=2 if st % 2 else 3, name="pp"
                )
                for it in range(n_it):
                    nc.tensor.matmul(
                        ps[:],
                        lhsT=xvt[it][:, st * 128:(st + 1) * 128],
                        rhs=wv[it][:],
                        start=(it == 0),
                        stop=(it == n_it - 1),
                    )
                nc.vector.tensor_copy(
                    Vp[:, st, :, :], ps.rearrange("p (h d) -> p h d", h=HPC)
                )
            return emit

        # prologue: all Vp groups (v arrives first on the wire), then the
        # first q/k projection slices; other slices are urgent sprinkles
        for st in range(16):
            v_proj_group(st)()
        qk_proj_group(QpT, wq, xq, 0, 0)
        qk_proj_group(KpT, wk, xk, 0, 0)

        # the n_it=9 (nonzero-bias) variant carries 12.5% more input data;
        # shrink the pipeline pools to fit SBUF (it is a correctness path)
        _exp_ctx = tc.tile_pool(name="expool", bufs=16 if n_it == 8 else 9)
        expool = _exp_ctx.__enter__()
        _hz_ctx = tc.tile_pool(name="hzpool", bufs=3 if n_it == 8 else 2)
        hzpool = _hz_ctx.__enter__()

        # Software pipeline: exp tiles wait in `pending` until their AV
        # matmuls are flushed a few steps later, so the PE never stalls
        # waiting for ScalarE and vice versa.
        pending = []  # entries: (et, kt, av_tile, hp, qh)

        def flush_one():
            et, kt, av, fhp, fqh = pending.pop(0)
            for j in range(2):
                nc.tensor.matmul(
                    av[j * D:(j + 1) * D, :],
                    lhsT=Vp[:, kt, 2 * fhp + j, :],
                    rhs=et[:, j * QW:(j + 1) * QW],
                    start=(kt == 0),
                    stop=(kt == 15),
                )
            if kt == 15:
                hz = hzpool.tile([128, QW], f32, tag="hz")
                nc.vector.tensor_copy(hz[:], av[:])
                nc.sync.dma_start(out=hzT[fhp, fqh], in_=hz[:])

        for hp in range(HPC // 2):
            sched = {}

            def put(step, unit):
                sched.setdefault(step, []).append(unit)

            if hp == 0:
                u = 0
                for sc in range(1, 4):  # KpT slices: needed from kt=4 on
                    for c in qk_proj_chunks(KpT, wk, xk, 0, sc):
                        put(u, c)
                        u += 1
                for sc in range(1, 4):  # QpT slices: needed from qh=sc on
                    for c in qk_proj_chunks(QpT, wq, xq, 0, sc):
                        put(u, c)
                        u += 1
                nxt = u
            else:
                nxt = 2
            if hp + 1 < HPC // 2:
                units = []
                for sc in range(4):
                    units += qk_proj_chunks(QpT, wq, xq, hp + 1, sc)
                    units += qk_proj_chunks(KpT, wk, xk, hp + 1, sc)
                span = NQH * 16 - nxt
                for i, c in enumerate(units):
                    put(nxt + (i * span) // len(units), c)
            depth = 4
            step = 0
            for qh in range(NQH):
                av = mmps.tile([128, QW], f32, tag="av", bufs=2, name="av")
                for kt2 in range(0, 16, 2):
                    qks = []
                    for kt in (kt2, kt2 + 1):
                        qk = mmps.tile(
                            [128, 2 * QW], f32, tag="qk", bufs=3, name="qk"
                        )
                        for j in range(2):
                            po = j * 64
                            nc.tensor.matmul(
                                qk[:, j * QW:(j + 1) * QW],
                                lhsT=KpT[po:po + 64, hp, kt * 128:(kt + 1) * 128],
                                rhs=QpT[po:po + 64, hp, qh * QW:(qh + 1) * QW],
                                start=True,
                                stop=True,
                            )
                        qks.append((qk, kt))
                    while len(pending) > depth:
                        flush_one()
                    for qk, kt in qks:
                        et = expool.tile([128, 2 * QW], dt, tag="exp")
                        nc.scalar.activation(et[:], qk[:], Exp)
                        nc.sync.dma_start(
                            out=expT[hp, qh, kt],
                            in_=et.rearrange("p (j w) -> p j w", j=2),
                        )
                        pending.append((et, kt, av, hp, qh))
                    for kt in (kt2, kt2 + 1):
                        for unit in sched.pop(step, []):
                            unit()
                        step += 1
            for s in sorted(sched):
                for unit in sched[s]:
                    unit()
            sched.clear()
        while pending:
            flush_one()

        _hz_ctx.__exit__(None, None, None)
        _exp_ctx.__exit__(None, None, None)

    nc.compile()
    return nc


def get_nc(n_it=8):
    key = ("nc", n_it)
    if key not in _CACHE:
        _CACHE[key] = _build_nc(n_it)
    return _CACHE[key]


def prepare_in_maps(q, k, v, Wq, bq, Wk, bk, Wv, bv, **_unused):
    dt = _np_dt()
    q, k, v = np.asarray(q), np.asarray(k), np.asarray(v)
    Wq, Wk, Wv = np.asarray(Wq), np.asarray(Wk), np.asarray(Wv)
    bq, bk, bv = np.asarray(bq), np.asarray(bk), np.asarray(bv)
    scale = 1.0 / np.sqrt(np.float32(D))
    zero_bias = not (np.any(bq) or np.any(bk) or np.any(bv))
    n_it = 8 if zero_bias else 9
    rows = n_it * 128

    def aug_x(xb):  # [S, HID] -> [rows, S]
        out = np.zeros((rows, S), dtype=dt)
        out[:HID] = xb.T.astype(dt)
        if n_it == 9:
            out[HID] = 1.0
        return out

    def aug_w(W, b, g, sc):  # -> [rows, CPC]
        out = np.zeros((rows, CPC), dtype=dt)
        sl = slice(g * CPC, (g + 1) * CPC)
        out[:HID] = (W[sl, :].T * sc).astype(dt)
        if n_it == 9:
            out[HID] = (b[sl] * sc).astype(dt)
        return out

    xcache = {b: (aug_x(q[b]), aug_x(k[b]), aug_x(v[b])) for b in range(B)}
    wcache = {
        g: (
            aug_w(Wq, bq, g, scale),
            aug_w(Wk, bk, g, 1.0),
            aug_w(Wv, bv, g, 1.0),
        )
        for g in range(2)
    }
    in_maps = []
    for c in range(N_CORES):
        b, g = c // 2, c % 2
        qTa, kTa, vTa = xcache[b]
        wqTa, wkTa, wvTa = wcache[g]
        in_maps.append(
            {"qT": qTa, "kT": kTa, "vT": vTa, "wqT": wqTa, "wkT": wkTa, "wvT": wvTa}
        )
    return in_maps, n_it


def assemble(results, mask=None):
    h = np.empty((B, S, HID), np.float32)
    scores = np.empty((B, H, S, S), np.float32)
    # additive -1e4 masking: exp(s-1e4) underflows to 0 in fp32, so masked
    # key columns just zero out; a fully-masked row is softmax(s - const)
    # = softmax(s), i.e. a no-op.  (The spec fixes mask = all ones.)
    mcol = [None] * B
    if mask is not None:
        mask = np.asarray(mask)
        for bb in range(B):
            mb = mask[bb]
            if np.any(mb == 0) and np.any(mb != 0):
                mcol[bb] = (mb != 0).astype(np.float32)
    for c in range(N_CORES):
        b, g = c // 2, c % 2
        hzt = np.asarray(results[c]["hzT"], np.float32)  # [HPC//2, NQH, 128, QW]
        hz = hzt.transpose(0, 2, 1, 3).reshape(HPC // 2, 128, S)
        expTc = results[c]["expT"]          # [HPC//2, NQH, 16, 128, 2, QW]
        for hl in range(HPC):
            hp, j = hl // 2, hl % 2
            e = np.asarray(expTc[hp, :, :, :, j], np.float32)  # [NQH,16,128,QW]
            e_kq = e.transpose(1, 2, 0, 3).reshape(S, S)    # [k, q]
            if mcol[b] is not None:
                e_kq = e_kq * mcol[b][:, None]
            Z = e_kq.sum(axis=0)                            # [q]
            scores[b, g * HPC + hl] = (e_kq / Z[None, :]).T
            h[b, :, g * CPC + hl * D:g * CPC + (hl + 1) * D] = (
                hz[hp, j * D:(j + 1) * D, :] / Z[None, :]
            ).T
    return h, scores


def run_spmd(in_maps, n_it=8, trace=False, tmpdir=None):
    from concourse.bass_utils import run_bass_kernel_spmd

    return run_bass_kernel_spmd(
        get_nc(n_it), in_maps, list(range(N_CORES)), trace=trace, tmpdir=tmpdir
    )


def kernel(**inputs):
    in_maps, n_it = prepare_in_maps(**inputs)
    res = run_spmd(in_maps, n_it)
    mask = inputs.get("mask")
    h, scores = assemble(res.results, mask)
    if mask is not None:
        mask = np.asarray(mask)
        for b in range(B):
            if np.any(mask[b] == 0) and np.any(mask[b] != 0):
                # device h used unmasked probs; redo this batch on host
                vp = (
                    np.asarray(inputs["v"][b], np.float32) @ np.asarray(inputs["Wv"], np.float32).T
                    + np.asarray(inputs["bv"], np.float32)
                )
                vh = vp.reshape(S, H, D).transpose(1, 0, 2)      # [H,S,D]
                hh = np.einsum("hqk,hkd->hqd", scores[b], vh)
                h[b] = hh.transpose(1, 0, 2).reshape(S, HID)
    return h, scores
